# revision 25
# baseline (speedup 1.0000x reference)
"""Causal self-attention (T=2048, C=2048, 16 heads) on 8 trn2 NeuronCores.

Sharding: tensor-parallel over heads — 2 heads per core. Each core computes
its QKV slice, attention for its 2 heads, and a partial output projection
(w_proj columns for its heads); the host sums the 8 partial outputs
(the "all-reduce" runs on host since outputs are gathered anyway).

The 8-core run is HBM-bandwidth-bound, so all tensors are re-laid-out on the
host into per-tile linear blocks (every DMA is a single contiguous 256KB
read/write) and partial outputs are written in fp16 (their magnitudes are
O(1); fp16 rounding of partials adds ~5e-4 relative error to the summed
output, well inside the f32r noise floor).

Math per core g (heads 2g, 2g+1), all matmuls in float32r (~tf32 precision):
  phase 1: qT/kT = (w_qk_g @ x.T)  laid out (head_dim, T) so scores can
           contract over head_dim on the partition axis; v = x @ w_v_g.T in
           natural (T, head_dim) layout for the PV contraction.
  phase 2: per 512-wide t-slice: scores_T tiles (s=128, t<=512) = kT_t.T @ qT,
           causal tile skipping (s_tile <= t_max) plus column skipping on the
           4 diagonal tiles (only t >= 128r is computed), exp on the scalar
           engine (scale=1/sqrt(hd) folded in), a 128x128 0/1 mask multiply on
           each diagonal block, PV with v stationary, softmax denominator via
           ones-stationary matmul, normalization through a rank-1 broadcast
           matmul of 1/den.
  phase 3: partial out = y_g @ w_proj_g.T, interleaved with phase 2 per slice.
"""

import math
import numpy as np

import concourse.bass as bass
import concourse.tile as tile
import concourse.mybir as mybir
from concourse.bass2jax import (
    _bass_exec_p,
    install_neuronx_cc_hook,
    partition_id_tensor,
)

T = 2048
C = 2048
H = 16
HD = 128          # head dim
G = 8             # cores
HPC = H // G      # heads per core = 2
D2 = HPC * HD     # 256 per-core q/k/v width
P = 128
TS = 512          # t-slice width
NSL = T // TS     # 4 slices
KC = C // P       # 16 contraction tiles
NT = T // P       # 16 t-tiles of 128
SQ = 1.0 / math.sqrt(HD)

F32 = mybir.dt.float32
F16 = mybir.dt.float16
R32 = mybir.dt.float32r


def _legalize_multiwaits(nc):
    """This container's walrus accepts one sync-wait per instruction; Tile's
    final drain carries several. Hoist extras onto preceding same-engine NOPs."""
    n = 0
    for f in nc.m.functions:
        for b in f.blocks:
            insts = list(b.instructions)
            out = []
            for inst in insts:
                si = inst.sync_info
                if si is not None and len(si.on_wait) > 1:
                    waits = list(si.on_wait)
                    for w in waits[:-1]:
                        nop = mybir.InstNoOp(name=f"legalize-nop-{n}", ins=[], outs=[])
                        n += 1
                        nop.engine = inst.engine
                        nop.sync_info = mybir.SyncInfo(on_wait=[w], on_update=[])
                        out.append(nop)
                    inst.sync_info = mybir.SyncInfo(
                        on_wait=[waits[-1]], on_update=list(si.on_update)
                    )
                out.append(inst)
            if len(out) != len(insts):
                b.instructions = out
    return n


def build_nc(reps=1):
    nc = bass.Bass("TRN2", target_bir_lowering=False, debug=False)

    # all inputs are pre-tiled on the host so each DMA below is one linear
    # 256KB (or smaller) contiguous block
    xt = nc.dram_tensor("xt", [KC, NSL, P, TS], R32, kind="ExternalInput").ap()
    wqk = nc.dram_tensor("wqk", [KC, P, 2 * D2], R32, kind="ExternalInput").ap()
    wv = nc.dram_tensor("wv", [KC, P, D2], R32, kind="ExternalInput").ap()
    wp = nc.dram_tensor("wp", [HPC, P, C], R32, kind="ExternalInput").ap()
    msk = nc.dram_tensor("msk", [P, P], R32, kind="ExternalInput").ap()
    out = nc.dram_tensor("out", [NT, 4, P, TS], F16, kind="ExternalOutput").ap()

    with tile.TileContext(nc) as tc:
        for _ in range(reps):
            _build_body(nc, tc, xt, wqk, wv, wp, msk, out)
    _legalize_multiwaits(nc)
    return nc


def _build_body(nc, tc, xt, wqk, wv, wp, msk, out):
    from contextlib import ExitStack

    with ExitStack() as ctx:
        sb = ctx.enter_context(tc.tile_pool(name="sb", bufs=1))
        ps = ctx.enter_context(tc.tile_pool(name="ps", bufs=1, space="PSUM"))

        masks = sb.tile([P, P], R32)
        nc.sync.dma_start(masks[:], msk[:])
        ones_f = sb.tile([P, P], F32)
        nc.vector.memset(ones_f[:], 1.0)
        ones = sb.tile([P, P], R32)
        nc.vector.tensor_copy(ones[:], ones_f[:])
        kT = sb.tile([P, HPC * T], R32)    # [k_h0 | k_h1] each (128, T)
        v_sb = sb.tile([P, NT * D2], R32)  # per t-tile: (128, 256) both heads
        wp_sb = sb.tile([P, HPC * C], R32)
        wqk_sb = sb.tile([P, KC * 2 * D2], R32)
        wv_sb = sb.tile([P, KC * D2], R32)

        def load_x(n):
            xh = []
            for k in range(KC):
                t_ = sb.tile([P, TS], R32, name=f"xh{n}_{k}", tag=f"xh{k}", bufs=1)
                nc.sync.dma_start(t_[:], xt[k, n])
                xh.append(t_)
            return xh

        # interleave the first x slice with the weight k-tiles so the first
        # matmul group isn't queued behind the full weight bulk
        xh_next = []
        for k in range(KC):
            t_ = sb.tile([P, TS], R32, name=f"xh0_{k}", tag=f"xh{k}", bufs=1)
            nc.sync.dma_start(t_[:], xt[k, 0])
            xh_next.append(t_)
            nc.sync.dma_start(wqk_sb[:, k * 2 * D2:(k + 1) * 2 * D2], wqk[k])
        for k in range(KC):
            nc.sync.dma_start(wv_sb[:, k * D2:(k + 1) * D2], wv[k])
        for h in range(HPC):
            nc.sync.dma_start(wp_sb[:, h * C:(h + 1) * C], wp[h])

        def phase1(n, xh):
            """QKV projections for t-slice n. Returns the rotating qT tiles."""
            qt = {}
            for j in range(4):
                # j: 0 -> q_h0, 1 -> q_h1, 2 -> k_h0, 3 -> k_h1
                psqk = ps.tile([P, TS], F32, name=f"psqk{n}_{j}", tag="p1", bufs=2)
                for k in range(KC):
                    nc.tensor.matmul(
                        psqk[:],
                        wqk_sb[:, k * 2 * D2 + j * P: k * 2 * D2 + (j + 1) * P],
                        xh[k][:],
                        start=(k == 0), stop=(k == KC - 1),
                    )
                if j < 2:
                    q_ = sb.tile([P, TS], R32, name=f"qt{n}_{j}",
                                 tag=f"qt{j}", bufs=2)
                    nc.scalar.copy(q_[:], psqk[:])
                    qt[j] = q_
                else:
                    h = j - 2
                    nc.scalar.copy(kT[:, h * T + n * TS: h * T + (n + 1) * TS],
                                   psqk[:])
            for m in range(4):
                psv = ps.tile([P, D2], F32, name=f"psv{n}_{m}", tag="p1", bufs=2)
                for k in range(KC):
                    nc.tensor.matmul(
                        psv[:],
                        xh[k][:, m * P:(m + 1) * P],
                        wv_sb[:, k * D2:(k + 1) * D2],
                        start=(k == 0), stop=(k == KC - 1),
                    )
                tt = 4 * n + m
                nc.scalar.copy(v_sb[:, tt * D2:(tt + 1) * D2], psv[:])
            return qt

        def attention(n, qt):
            """Causal softmax attention for t-slice n; returns yT tiles."""
            nsig = 4 * n + 4   # kept s-tiles
            ytile = {}
            for h in range(HPC):
                es = []
                for s in range(nsig):
                    r = s - 4 * n  # >=0 on the 4 diagonal tiles
                    lo = 128 * r if r > 0 else 0  # computed t-range start
                    psc = ps.tile([P, TS], F32, name=f"sc{n}_{h}_{s}",
                                  tag="psA", bufs=2)
                    nc.tensor.matmul(
                        psc[:, lo:],
                        kT[:, h * T + s * P: h * T + (s + 1) * P],
                        qt[h][:, lo:],
                        start=True, stop=True,
                    )
                    e = sb.tile([P, TS], R32, name=f"e{n}_{h}_{s}",
                                tag=f"e{h}", bufs=9)
                    nc.scalar.activation(
                        e[:, lo:], psc[:, lo:],
                        mybir.ActivationFunctionType.Exp, scale=SQ,
                    )
                    if r >= 0:
                        nc.vector.tensor_mul(
                            e[:, 128 * r:128 * (r + 1)],
                            e[:, 128 * r:128 * (r + 1)],
                            masks[:],
                        )
                    es.append((e, lo))
                psy = ps.tile([P, TS], F32, name=f"psy{n}_{h}", tag="psy", bufs=2)
                psden = ps.tile([1, TS], F32, name=f"psden{n}_{h}",
                                tag="psmall", bufs=2)
                for s in range(nsig):
                    e, lo = es[s]
                    nc.tensor.matmul(
                        psy[:, lo:],
                        v_sb[:, s * D2 + h * P: s * D2 + (h + 1) * P],
                        e[:, lo:],
                        start=(s == 0), stop=(s == nsig - 1),
                    )
                    nc.tensor.matmul(
                        psden[:, lo:], ones[:, 0:1], e[:, lo:],
                        start=(s == 0), stop=(s == nsig - 1),
                    )
                rcp = sb.tile([1, TS], R32, name=f"rcp{n}_{h}", tag="rcp", bufs=2)
                with nc.allow_low_precision(reason="f32r output for broadcast"):
                    nc.vector.reciprocal(rcp[:], psden[:])
                psb = ps.tile([P, TS], F32, name=f"psb{n}_{h}",
                              tag="psmall", bufs=2)
                nc.tensor.matmul(psb[:], ones[0:1, :], rcp[:],
                                 start=True, stop=True)
                bsb = sb.tile([P, TS], R32, name=f"bsb{n}_{h}", tag="bsb", bufs=2)
                nc.scalar.copy(bsb[:], psb[:])
                yt = sb.tile([P, TS], R32, name=f"yT{n}_{h}", tag=f"yT{h}", bufs=2)
                nc.vector.tensor_mul(yt[:], psy[:], bsb[:])
                ytile[h] = yt
            return ytile

        def proj(n, ytile):
            """Partial output projection for the 4 t-tiles of slice n."""
            for m in range(4):
                tt = 4 * n + m
                for u in range(4):
                    pso = ps.tile([P, TS], F32, name=f"pso{tt}_{u}",
                                  tag="p1", bufs=2)
                    for h in range(HPC):
                        nc.tensor.matmul(
                            pso[:],
                            ytile[h][:, m * P:(m + 1) * P],
                            wp_sb[:, h * C + u * TS: h * C + (u + 1) * TS],
                            start=(h == 0), stop=(h == HPC - 1),
                        )
                    osb = sb.tile([P, TS], F16, name=f"osb{tt}_{u}",
                                  tag="osb", bufs=3)
                    nc.vector.tensor_copy(osb[:], pso[:])
                    nc.sync.dma_start(out[tt, u], osb[:])

        qt = phase1(0, xh_next)
        for n in range(NSL):
            if n + 1 < NSL:
                xh_next = load_x(n + 1)
            ytile = attention(n, qt)
            proj(n, ytile)
            if n + 1 < NSL:
                qt = phase1(n + 1, xh_next)


# ---------------------------------------------------------------------------
# host-side: sharding, runner, gather

class _Runner:
    """Jit once, run many. Mirrors bass2jax.run_bass_via_pjrt's multi-core path."""

    def __init__(self, nc, n_cores):
        import jax
        from jax.sharding import Mesh, PartitionSpec
        from jax.experimental.shard_map import shard_map

        install_neuronx_cc_hook()
        self.n_cores = n_cores
        partition_name = (
            nc.partition_id_tensor.name if nc.partition_id_tensor else None
        )
        in_names, out_names, out_avals, zero_outs = [], [], [], []
        for alloc in nc.m.functions[0].allocations:
            if not isinstance(alloc, mybir.MemoryLocationSet):
                continue
            name = alloc.memorylocations[0].name
            if alloc.kind == "ExternalInput":
                if name != partition_name:
                    in_names.append(name)
            elif alloc.kind == "ExternalOutput":
                shape = tuple(alloc.tensor_shape)
                dtype = mybir.dt.np(alloc.dtype)
                out_avals.append(jax.core.ShapedArray(shape, dtype))
                out_names.append(name)
                zero_outs.append(np.zeros(shape, dtype))
        self.in_names, self.out_names = in_names, out_names
        self.out_avals, self.zero_outs = out_avals, zero_outs
        n_params, n_outs = len(in_names), len(out_names)
        bind_in_names = list(in_names) + list(out_names)
        if partition_name is not None:
            bind_in_names.append(partition_name)

        def _body(*args):
            operands = list(args)
            if partition_name is not None:
                operands.append(partition_id_tensor())
            outs = _bass_exec_p.bind(
                *operands,
                out_avals=tuple(out_avals),
                in_names=tuple(bind_in_names),
                out_names=tuple(out_names),
                lowering_input_output_aliases=(),
                sim_require_finite=True,
                sim_require_nnan=True,
                nc=nc,
            )
            return tuple(outs)

        devices = jax.devices()[:n_cores]
        assert len(devices) == n_cores, (
            f"need {n_cores} neuron cores, found {len(jax.devices())}"
        )
        mesh = Mesh(np.asarray(devices), ("core",))
        # inputs identical on every core are replicated instead of concatenated
        self.replicated = {"xt", "msk"}
        in_specs = tuple(
            PartitionSpec() if nm in self.replicated else PartitionSpec("core")
            for nm in in_names
        ) + (PartitionSpec("core"),) * n_outs
        out_specs = (PartitionSpec("core"),) * n_outs
        self._fn = jax.jit(
            shard_map(_body, mesh=mesh, in_specs=in_specs,
                      out_specs=out_specs, check_rep=False),
            keep_unused=True,
        )
        self._jax = jax
        self._mesh = mesh
        # zero output buffers never change: upload once, reuse every call
        self._dev_zeros = None

    def run(self, in_maps):
        import jax
        from jax.sharding import NamedSharding, PartitionSpec

        n = self.n_cores
        if self._dev_zeros is None:
            sh = NamedSharding(self._mesh, PartitionSpec("core"))
            self._dev_zeros = [
                jax.device_put(
                    np.zeros((n * z.shape[0], *z.shape[1:]), z.dtype), sh
                )
                for z in self.zero_outs
            ]
        args = []
        for i, name in enumerate(self.in_names):
            if name in self.replicated:
                args.append(np.ascontiguousarray(in_maps[0][name]))
            else:
                args.append(np.concatenate(
                    [np.ascontiguousarray(m[name]) for m in in_maps], axis=0
                ))
        outs = self._fn(*args, *self._dev_zeros)
        return [
            {name: np.asarray(outs[i]).reshape(n, *self.out_avals[i].shape)[c]
             for i, name in enumerate(self.out_names)}
            for c in range(n)
        ]


_RUNNER = None


def shard_inputs(x, w_attn, w_proj):
    """Full inputs -> list of 8 per-core input dicts (pre-tiled layouts)."""
    # x.T tiled: [k, n, p, ts]
    xt = np.ascontiguousarray(
        x.T.reshape(KC, P, NSL, TS).transpose(0, 2, 1, 3)
    )
    mask = np.triu(np.ones((P, P), dtype=np.float32))  # keep where t >= s
    in_maps = []
    for g in range(G):
        qs, ks, vs = D2 * g, C + D2 * g, 2 * C + D2 * g
        wqk_g = np.concatenate([w_attn[qs:qs + D2], w_attn[ks:ks + D2]], axis=0).T
        wqk_g = np.ascontiguousarray(wqk_g.reshape(KC, P, 2 * D2))
        wv_g = np.ascontiguousarray(w_attn[vs:vs + D2].T.reshape(KC, P, D2))
        wp_g = np.ascontiguousarray(
            w_proj[:, D2 * g:D2 * (g + 1)].T.reshape(HPC, P, C)
        )
        in_maps.append({
            "xt": xt, "wqk": wqk_g, "wv": wv_g, "wp": wp_g, "msk": mask,
        })
    return in_maps


def kernel(x, w_attn, w_proj):
    global _RUNNER
    x = np.asarray(x, dtype=np.float32)
    w_attn = np.asarray(w_attn, dtype=np.float32)
    w_proj = np.asarray(w_proj, dtype=np.float32)
    if _RUNNER is None:
        _RUNNER = _Runner(build_nc(), G)
    results = _RUNNER.run(shard_inputs(x, w_attn, w_proj))
    acc = np.zeros((T, C), dtype=np.float32)
    for g in range(G):
        o = results[g]["out"]  # (NT, 4, P, TS) fp16
        acc += o.transpose(0, 2, 1, 3).reshape(T, C).astype(np.float32)
    return acc


# revision 26
# speedup vs baseline: 1.4962x; 1.4962x over previous
"""Causal self-attention (T=2048, C=2048, 16 heads) on 8 trn2 NeuronCores.

Sharding: tensor-parallel over heads — 2 heads per core. Each core computes
its QKV slice, attention for its 2 heads, and a partial output projection
(w_proj columns for its heads); the host sums the 8 partial outputs
(the "all-reduce" runs on host since outputs are gathered anyway).

The 8-core run is HBM-bandwidth-bound, so all tensors are re-laid-out on the
host into per-tile linear blocks (every DMA is a single contiguous 256KB
read/write) and partial outputs are written in fp16 (their magnitudes are
O(1); fp16 rounding of partials adds ~5e-4 relative error to the summed
output, well inside the f32r noise floor).

Math per core g (heads 2g, 2g+1), all matmuls in float32r (~tf32 precision):
  phase 1: qT/kT = (w_qk_g @ x.T)  laid out (head_dim, T) so scores can
           contract over head_dim on the partition axis; v = x @ w_v_g.T in
           natural (T, head_dim) layout for the PV contraction.
  phase 2: per 512-wide t-slice: scores_T tiles (s=128, t<=512) = kT_t.T @ qT,
           causal tile skipping (s_tile <= t_max) plus column skipping on the
           4 diagonal tiles (only t >= 128r is computed), exp on the scalar
           engine (scale=1/sqrt(hd) folded in), a 128x128 0/1 mask multiply on
           each diagonal block, PV with v stationary, softmax denominator via
           ones-stationary matmul, normalization through a rank-1 broadcast
           matmul of 1/den.
  phase 3: partial out = y_g @ w_proj_g.T, interleaved with phase 2 per slice.
"""

import math
import numpy as np

import concourse.bass as bass
import concourse.tile as tile
import concourse.mybir as mybir
from concourse.bass2jax import (
    _bass_exec_p,
    install_neuronx_cc_hook,
    partition_id_tensor,
)

T = 2048
C = 2048
H = 16
HD = 128          # head dim
G = 8             # cores
HPC = H // G      # heads per core = 2
D2 = HPC * HD     # 256 per-core q/k/v width
P = 128
TS = 512          # t-slice width
NSL = T // TS     # 4 slices
KC = C // P       # 16 contraction tiles
NT = T // P       # 16 t-tiles of 128
SQ = 1.0 / math.sqrt(HD)

F32 = mybir.dt.float32
F16 = mybir.dt.float16
R32 = mybir.dt.float32r


def _legalize_multiwaits(nc):
    """This container's walrus accepts one sync-wait per instruction; Tile's
    final drain carries several. Hoist extras onto preceding same-engine NOPs."""
    n = 0
    for f in nc.m.functions:
        for b in f.blocks:
            insts = list(b.instructions)
            out = []
            for inst in insts:
                si = inst.sync_info
                if si is not None and len(si.on_wait) > 1:
                    waits = list(si.on_wait)
                    for w in waits[:-1]:
                        nop = mybir.InstNoOp(name=f"legalize-nop-{n}", ins=[], outs=[])
                        n += 1
                        nop.engine = inst.engine
                        nop.sync_info = mybir.SyncInfo(on_wait=[w], on_update=[])
                        out.append(nop)
                    inst.sync_info = mybir.SyncInfo(
                        on_wait=[waits[-1]], on_update=list(si.on_update)
                    )
                out.append(inst)
            if len(out) != len(insts):
                b.instructions = out
    return n


def build_nc(reps=1):
    nc = bass.Bass("TRN2", target_bir_lowering=False, debug=False)

    # all inputs are pre-tiled on the host so each DMA below is one linear
    # 256KB (or smaller) contiguous block
    xt = nc.dram_tensor("xt", [KC, NSL, P, TS], F16, kind="ExternalInput").ap()
    wqk = nc.dram_tensor("wqk", [KC, P, 2 * D2], F16, kind="ExternalInput").ap()
    wv = nc.dram_tensor("wv", [KC, P, D2], F16, kind="ExternalInput").ap()
    wp = nc.dram_tensor("wp", [HPC, P, C], F16, kind="ExternalInput").ap()
    msk = nc.dram_tensor("msk", [P, P], R32, kind="ExternalInput").ap()
    out = nc.dram_tensor("out", [NT, 4, P, TS], F16, kind="ExternalOutput").ap()

    with tile.TileContext(nc) as tc:
        for _ in range(reps):
            _build_body(nc, tc, xt, wqk, wv, wp, msk, out)
    _legalize_multiwaits(nc)
    return nc


def _build_body(nc, tc, xt, wqk, wv, wp, msk, out):
    from contextlib import ExitStack

    with ExitStack() as ctx:
        sb = ctx.enter_context(tc.tile_pool(name="sb", bufs=1))
        ps = ctx.enter_context(tc.tile_pool(name="ps", bufs=1, space="PSUM"))

        masks = sb.tile([P, P], R32)
        nc.sync.dma_start(masks[:], msk[:])
        ones_f = sb.tile([P, P], F32)
        nc.vector.memset(ones_f[:], 1.0)
        ones = sb.tile([P, P], R32)
        nc.vector.tensor_copy(ones[:], ones_f[:])
        kT = sb.tile([P, HPC * T], R32)    # [k_h0 | k_h1] each (128, T)
        v_sb = sb.tile([P, NT * D2], R32)  # per t-tile: (128, 256) both heads
        wp_sb = sb.tile([P, HPC * C], F16)
        wqk_sb = sb.tile([P, KC * 2 * D2], F16)
        wv_sb = sb.tile([P, KC * D2], F16)

        def load_x(n):
            xh = []
            for k in range(KC):
                t_ = sb.tile([P, TS], F16, name=f"xh{n}_{k}", tag=f"xh{k}", bufs=1)
                nc.sync.dma_start(t_[:], xt[k, n])
                xh.append(t_)
            return xh

        # interleave the first x slice with the weight k-tiles so the first
        # matmul group isn't queued behind the full weight bulk
        xh_next = []
        for k in range(KC):
            t_ = sb.tile([P, TS], F16, name=f"xh0_{k}", tag=f"xh{k}", bufs=1)
            nc.sync.dma_start(t_[:], xt[k, 0])
            xh_next.append(t_)
            nc.sync.dma_start(wqk_sb[:, k * 2 * D2:(k + 1) * 2 * D2], wqk[k])
        for k in range(KC):
            nc.sync.dma_start(wv_sb[:, k * D2:(k + 1) * D2], wv[k])
        for h in range(HPC):
            nc.sync.dma_start(wp_sb[:, h * C:(h + 1) * C], wp[h])

        def phase1(n, xh):
            """QKV projections for t-slice n. Returns the rotating qT tiles."""
            qt = {}
            for j in range(4):
                # j: 0 -> q_h0, 1 -> q_h1, 2 -> k_h0, 3 -> k_h1
                psqk = ps.tile([P, TS], F32, name=f"psqk{n}_{j}", tag="p1", bufs=2)
                for k in range(KC):
                    nc.tensor.matmul(
                        psqk[:],
                        wqk_sb[:, k * 2 * D2 + j * P: k * 2 * D2 + (j + 1) * P],
                        xh[k][:],
                        start=(k == 0), stop=(k == KC - 1),
                    )
                if j < 2:
                    q_ = sb.tile([P, TS], R32, name=f"qt{n}_{j}",
                                 tag=f"qt{j}", bufs=2)
                    nc.scalar.copy(q_[:], psqk[:])
                    qt[j] = q_
                else:
                    h = j - 2
                    nc.scalar.copy(kT[:, h * T + n * TS: h * T + (n + 1) * TS],
                                   psqk[:])
            for m in range(4):
                psv = ps.tile([P, D2], F32, name=f"psv{n}_{m}", tag="p1", bufs=2)
                for k in range(KC):
                    nc.tensor.matmul(
                        psv[:],
                        xh[k][:, m * P:(m + 1) * P],
                        wv_sb[:, k * D2:(k + 1) * D2],
                        start=(k == 0), stop=(k == KC - 1),
                    )
                tt = 4 * n + m
                nc.scalar.copy(v_sb[:, tt * D2:(tt + 1) * D2], psv[:])
            return qt

        def attention(n, qt):
            """Causal softmax attention for t-slice n; returns yT tiles."""
            nsig = 4 * n + 4   # kept s-tiles
            ytile = {}
            for h in range(HPC):
                es = []
                for s in range(nsig):
                    r = s - 4 * n  # >=0 on the 4 diagonal tiles
                    lo = 128 * r if r > 0 else 0  # computed t-range start
                    psc = ps.tile([P, TS], F32, name=f"sc{n}_{h}_{s}",
                                  tag="psA", bufs=2)
                    nc.tensor.matmul(
                        psc[:, lo:],
                        kT[:, h * T + s * P: h * T + (s + 1) * P],
                        qt[h][:, lo:],
                        start=True, stop=True,
                    )
                    e = sb.tile([P, TS], R32, name=f"e{n}_{h}_{s}",
                                tag=f"e{h}", bufs=16)
                    nc.scalar.activation(
                        e[:, lo:], psc[:, lo:],
                        mybir.ActivationFunctionType.Exp, scale=SQ,
                    )
                    if r >= 0:
                        nc.vector.tensor_mul(
                            e[:, 128 * r:128 * (r + 1)],
                            e[:, 128 * r:128 * (r + 1)],
                            masks[:],
                        )
                    es.append((e, lo))
                psy = ps.tile([P, TS], F32, name=f"psy{n}_{h}", tag="psy", bufs=2)
                psden = ps.tile([1, TS], F32, name=f"psden{n}_{h}",
                                tag="psmall", bufs=2)
                for s in range(nsig):
                    e, lo = es[s]
                    nc.tensor.matmul(
                        psy[:, lo:],
                        v_sb[:, s * D2 + h * P: s * D2 + (h + 1) * P],
                        e[:, lo:],
                        start=(s == 0), stop=(s == nsig - 1),
                    )
                    nc.tensor.matmul(
                        psden[:, lo:], ones[:, 0:1], e[:, lo:],
                        start=(s == 0), stop=(s == nsig - 1),
                    )
                rcp = sb.tile([1, TS], R32, name=f"rcp{n}_{h}", tag="rcp", bufs=2)
                with nc.allow_low_precision(reason="f32r output for broadcast"):
                    nc.vector.reciprocal(rcp[:], psden[:])
                psb = ps.tile([P, TS], F32, name=f"psb{n}_{h}",
                              tag="psmall", bufs=2)
                nc.tensor.matmul(psb[:], ones[0:1, :], rcp[:],
                                 start=True, stop=True)
                bsb = sb.tile([P, TS], R32, name=f"bsb{n}_{h}", tag="bsb", bufs=2)
                nc.scalar.copy(bsb[:], psb[:])
                yt = sb.tile([P, TS], F16, name=f"yT{n}_{h}", tag=f"yT{h}", bufs=2)
                nc.vector.tensor_mul(yt[:], psy[:], bsb[:])
                ytile[h] = yt
            return ytile

        def proj(n, ytile):
            """Partial output projection for the 4 t-tiles of slice n."""
            for m in range(4):
                tt = 4 * n + m
                for u in range(4):
                    pso = ps.tile([P, TS], F32, name=f"pso{tt}_{u}",
                                  tag="p1", bufs=2)
                    for h in range(HPC):
                        nc.tensor.matmul(
                            pso[:],
                            ytile[h][:, m * P:(m + 1) * P],
                            wp_sb[:, h * C + u * TS: h * C + (u + 1) * TS],
                            start=(h == 0), stop=(h == HPC - 1),
                        )
                    osb = sb.tile([P, TS], F16, name=f"osb{tt}_{u}",
                                  tag="osb", bufs=3)
                    nc.vector.tensor_copy(osb[:], pso[:])
                    nc.sync.dma_start(out[tt, u], osb[:])

        qt = phase1(0, xh_next)
        for n in range(NSL):
            if n + 1 < NSL:
                xh_next = load_x(n + 1)
            ytile = attention(n, qt)
            proj(n, ytile)
            if n + 1 < NSL:
                qt = phase1(n + 1, xh_next)


# ---------------------------------------------------------------------------
# host-side: sharding, runner, gather

class _Runner:
    """Jit once, run many. Mirrors bass2jax.run_bass_via_pjrt's multi-core path."""

    def __init__(self, nc, n_cores):
        import jax
        from jax.sharding import Mesh, PartitionSpec
        from jax.experimental.shard_map import shard_map

        install_neuronx_cc_hook()
        self.n_cores = n_cores
        partition_name = (
            nc.partition_id_tensor.name if nc.partition_id_tensor else None
        )
        in_names, out_names, out_avals, zero_outs = [], [], [], []
        for alloc in nc.m.functions[0].allocations:
            if not isinstance(alloc, mybir.MemoryLocationSet):
                continue
            name = alloc.memorylocations[0].name
            if alloc.kind == "ExternalInput":
                if name != partition_name:
                    in_names.append(name)
            elif alloc.kind == "ExternalOutput":
                shape = tuple(alloc.tensor_shape)
                dtype = mybir.dt.np(alloc.dtype)
                out_avals.append(jax.core.ShapedArray(shape, dtype))
                out_names.append(name)
                zero_outs.append(np.zeros(shape, dtype))
        self.in_names, self.out_names = in_names, out_names
        self.out_avals, self.zero_outs = out_avals, zero_outs
        n_params, n_outs = len(in_names), len(out_names)
        bind_in_names = list(in_names) + list(out_names)
        if partition_name is not None:
            bind_in_names.append(partition_name)

        def _body(*args):
            operands = list(args)
            if partition_name is not None:
                operands.append(partition_id_tensor())
            outs = _bass_exec_p.bind(
                *operands,
                out_avals=tuple(out_avals),
                in_names=tuple(bind_in_names),
                out_names=tuple(out_names),
                lowering_input_output_aliases=(),
                sim_require_finite=True,
                sim_require_nnan=True,
                nc=nc,
            )
            return tuple(outs)

        devices = jax.devices()[:n_cores]
        assert len(devices) == n_cores, (
            f"need {n_cores} neuron cores, found {len(jax.devices())}"
        )
        mesh = Mesh(np.asarray(devices), ("core",))
        # inputs identical on every core are replicated instead of concatenated
        self.replicated = {"xt", "msk"}
        in_specs = tuple(
            PartitionSpec() if nm in self.replicated else PartitionSpec("core")
            for nm in in_names
        ) + (PartitionSpec("core"),) * n_outs
        out_specs = (PartitionSpec("core"),) * n_outs
        self._fn = jax.jit(
            shard_map(_body, mesh=mesh, in_specs=in_specs,
                      out_specs=out_specs, check_rep=False),
            keep_unused=True,
        )
        self._jax = jax
        self._mesh = mesh
        # zero output buffers never change: upload once, reuse every call
        self._dev_zeros = None

    def run(self, in_maps):
        import jax
        from jax.sharding import NamedSharding, PartitionSpec

        n = self.n_cores
        if self._dev_zeros is None:
            sh = NamedSharding(self._mesh, PartitionSpec("core"))
            self._dev_zeros = [
                jax.device_put(
                    np.zeros((n * z.shape[0], *z.shape[1:]), z.dtype), sh
                )
                for z in self.zero_outs
            ]
        args = []
        for i, name in enumerate(self.in_names):
            if name in self.replicated:
                args.append(np.ascontiguousarray(in_maps[0][name]))
            else:
                args.append(np.concatenate(
                    [np.ascontiguousarray(m[name]) for m in in_maps], axis=0
                ))
        outs = self._fn(*args, *self._dev_zeros)
        return [
            {name: np.asarray(outs[i]).reshape(n, *self.out_avals[i].shape)[c]
             for i, name in enumerate(self.out_names)}
            for c in range(n)
        ]


_RUNNER = None


def shard_inputs(x, w_attn, w_proj):
    """Full inputs -> list of 8 per-core input dicts (pre-tiled layouts)."""
    # x.T tiled: [k, n, p, ts]
    xt = np.ascontiguousarray(
        x.T.reshape(KC, P, NSL, TS).transpose(0, 2, 1, 3)
    ).astype(np.float16)
    mask = np.triu(np.ones((P, P), dtype=np.float32))  # keep where t >= s
    in_maps = []
    for g in range(G):
        qs, ks, vs = D2 * g, C + D2 * g, 2 * C + D2 * g
        wqk_g = np.concatenate([w_attn[qs:qs + D2], w_attn[ks:ks + D2]], axis=0).T
        wqk_g = np.ascontiguousarray(wqk_g.reshape(KC, P, 2 * D2)).astype(np.float16)
        wv_g = np.ascontiguousarray(w_attn[vs:vs + D2].T.reshape(KC, P, D2)).astype(np.float16)
        wp_g = np.ascontiguousarray(
            w_proj[:, D2 * g:D2 * (g + 1)].T.reshape(HPC, P, C)
        ).astype(np.float16)
        in_maps.append({
            "xt": xt, "wqk": wqk_g, "wv": wv_g, "wp": wp_g, "msk": mask,
        })
    return in_maps


def kernel(x, w_attn, w_proj):
    global _RUNNER
    x = np.asarray(x, dtype=np.float32)
    w_attn = np.asarray(w_attn, dtype=np.float32)
    w_proj = np.asarray(w_proj, dtype=np.float32)
    if _RUNNER is None:
        _RUNNER = _Runner(build_nc(), G)
    results = _RUNNER.run(shard_inputs(x, w_attn, w_proj))
    acc = np.zeros((T, C), dtype=np.float32)
    for g in range(G):
        o = results[g]["out"]  # (NT, 4, P, TS) fp16
        acc += o.transpose(0, 2, 1, 3).reshape(T, C).astype(np.float32)
    return acc
